# revision 8
# baseline (speedup 1.0000x reference)
"""Trainium2 8-core Bass kernel for nn_AttentionHPROJ (B=2,T=2048,C=1024,h=16,hd=64).

Sharding: core = 4*b + g owns batch b and heads [4g, 4g+4).
All compute in transposed layout (dout on partitions, tokens moving).

v3: fp16/bf16 matmul dataflow (PSUM accumulation fp32):
  - Q^T/K^T = wqk^T @ x^T in fp16; V natural = x @ Wv^T -> bf16.
  - S^T per head pair packed in one [128,2,512] PSUM tile; h-even on PE rows
    0:64, h-odd on rows 64:128 (concurrent quadrant execution, ~2x).
  - One exp (ACT) per head PAIR over [128,2,width] -> w_pair bf16; causal
    mask multiply on diagonal subtiles (DVE).
  - AV: O^T per head in [65,512] PSUM, ones-column softmax denominator.
  - Normalize: copy denom row to SBUF, reciprocal_approx_fast (the custom op
    NaNs when reading PSUM directly), gpsimd partition_broadcast, DVE mult.
  - c_proj fp16 -> partial Y^T per core, fp16 out; host sums quad + transposes.
  - All proj/c_proj matmul chains run as interleaved PAIRS (alternating PSUM
    banks) so back-to-back matmuls pipeline (~218ns vs ~430ns each).
  - Inputs host-relayouted to SBUF layout: every DMA is [128 part x contiguous
    bytes]; bulk tensors land in 1-2 DMAs; y-out DMAs dispatched from gpsimd.
"""
import sys

sys.path.insert(0, "/opt/trn_rl_repo")

import numpy as np

B, T, C = 2, 2048, 1024
NH, HD = 16, 64
P = 128
NCORE = 8
QC = 512          # q-chunk width
NQC = T // QC     # 4
KSUB = C // P     # 8

_CACHE = {}


def _build():
    import functools
    import concourse.bass as bass
    import concourse.mybir as mybir
    import concourse.tile as tile
    from concourse import bacc

    F32 = mybir.dt.float32
    F16 = mybir.dt.float16
    BF16 = mybir.dt.bfloat16

    import time as _time
    _t0 = _time.time()
    print("[build] start v3", flush=True)
    nc = bacc.Bacc("TRN2", target_bir_lowering=False, debug=False, num_devices=NCORE)

    xt_ext = nc.dram_tensor("xt", [NQC, P, KSUB, QC], F16, kind="ExternalInput").ap()
    wqk_ext = nc.dram_tensor("wqk", [P, KSUB, 512], F16, kind="ExternalInput").ap()
    wv_ext = nc.dram_tensor("wv", [P, KSUB, 256], F16, kind="ExternalInput").ap()
    wp_ext = nc.dram_tensor("wp", [P, 2, C], F16, kind="ExternalInput").ap()
    mask_ext = nc.dram_tensor("mask", [P, 2, P], F32, kind="ExternalInput").ap()
    out_ext = nc.dram_tensor("out", [KSUB, NQC, P, QC], F16,
                             kind="ExternalOutput").ap()

    with tile.TileContext(nc) as tc:
        with (
            tc.tile_pool(name="wpool", bufs=1) as wpool,
            tc.tile_pool(name="xpool", bufs=1) as xpool,
            tc.tile_pool(name="qkpool", bufs=1) as qkpool,
            tc.tile_pool(name="vpool", bufs=1) as vpool,
            tc.tile_pool(name="wtile", bufs=4) as wtpool,
            tc.tile_pool(name="opool", bufs=2) as opool,
            tc.tile_pool(name="small", bufs=4) as small,
            tc.tile_pool(name="psS", bufs=2, space="PSUM") as psS,
            tc.tile_pool(name="psO", bufs=2, space="PSUM") as psO,
            tc.tile_pool(name="psY", bufs=2, space="PSUM") as psY,
        ):
            # ---- inputs ----
            wqk_sb = wpool.tile([P, KSUB, 512], F16)
            wv_sb = wpool.tile([P, KSUB, 256], F16)
            xt_t = [xpool.tile([P, KSUB, QC], F16, name=f"xt_{i}", tag=f"xt_{i}")
                    for i in range(NQC)]
            qk_t = [qkpool.tile([P, 4, QC], F16, name=f"qk_{i}", tag=f"qk_{i}")
                    for i in range(NQC)]
            v_t = [vpool.tile([P, 4, 4 * 65], BF16, name=f"v_{i}", tag=f"v_{i}")
                   for i in range(NQC)]
            mask_sb = wpool.tile([P, 2, P], F32)
            wp_sb = wpool.tile([P, 2, C], F16)

            nc.sync.dma_start(mask_sb[:], mask_ext[:])
            ones_st = wpool.tile([P, 4, 4], BF16)
            nc.vector.memset(ones_st[:], 1.0)
            for i in range(NQC):
                ov = v_t[i].rearrange("p t (h c) -> p t h c", c=65)[:, :, :, 64]
                nc.vector.tensor_copy(out=ov, in_=ones_st[:])
            # early-need inputs on sync, halves for low first-matmul latency
            nc.sync.dma_start(wqk_sb[:, 0:4], wqk_ext[:, 0:4])
            nc.sync.dma_start(xt_t[0][:, 0:4], xt_ext[0, :, 0:4])
            nc.sync.dma_start(wv_sb[:], wv_ext[:])
            nc.sync.dma_start(wqk_sb[:, 4:8], wqk_ext[:, 4:8])
            nc.sync.dma_start(xt_t[0][:, 4:8], xt_ext[0, :, 4:8])
            nc.sync.dma_start(wp_sb[:], wp_ext[:])
            for i in range(1, NQC):
                nc.sync.dma_start(xt_t[i][:], xt_ext[i])

            # ---- interleaved-pair projection chains ----
            def qk_pair_step(tc_i, dtA, dtB, st, k):
                if k == 0:
                    st["a"] = psY.tile([P, QC], F32, name=f"pqk{tc_i}{dtA}", tag="Y")
                    st["b"] = psY.tile([P, QC], F32, name=f"pqk{tc_i}{dtB}", tag="Y")
                for ps, dt in ((st["a"], dtA), (st["b"], dtB)):
                    nc.tensor.matmul(
                        ps[:],
                        lhsT=wqk_sb[:, k, dt * P : (dt + 1) * P],
                        rhs=xt_t[tc_i][:, k],
                        start=(k == 0),
                        stop=(k == KSUB - 1),
                    )
                if k == KSUB - 1:
                    nc.vector.tensor_copy(out=qk_t[tc_i][:, dtA, :], in_=st["a"][:])
                    nc.vector.tensor_copy(out=qk_t[tc_i][:, dtB, :], in_=st["b"][:])

            def v_pair_step(tc_i, tlA, tlB, st, k):
                if k == 0:
                    st["a"] = psY.tile([P, 256], F32, name=f"pv{tc_i}{tlA}", tag="Y")
                    st["b"] = psY.tile([P, 256], F32, name=f"pv{tc_i}{tlB}", tag="Y")
                for ps, tl_ in ((st["a"], tlA), (st["b"], tlB)):
                    nc.tensor.matmul(
                        ps[:],
                        lhsT=xt_t[tc_i][:, k, tl_ * P : (tl_ + 1) * P],
                        rhs=wv_sb[:, k, :],
                        start=(k == 0),
                        stop=(k == KSUB - 1),
                    )
                if k == KSUB - 1:
                    for ps, tl_ in ((st["a"], tlA), (st["b"], tlB)):
                        vdst = v_t[tc_i].rearrange(
                            "p t (h c) -> p t h c", c=65)[:, tl_, :, 0:64]
                        nc.vector.tensor_copy(
                            out=vdst, in_=ps.rearrange("p (h d) -> p h d", d=64)
                        )

            def pair_unit(fn, *args):
                st = {}
                return [functools.partial(fn, *args, st, k) for k in range(KSUB)]

            def cproj_steps(qc, o_t, dtA, dtB):
                st = {}

                def step(pr):
                    if pr == 0:
                        st["a"] = psY.tile([P, QC], F32, name=f"py{qc}{dtA}", tag="Y")
                        st["b"] = psY.tile([P, QC], F32, name=f"py{qc}{dtB}", tag="Y")
                    for ps, dt in ((st["a"], dtA), (st["b"], dtB)):
                        nc.tensor.matmul(
                            ps[:],
                            lhsT=wp_sb[:, pr, dt * P : (dt + 1) * P],
                            rhs=o_t[:, pr, :],
                            start=(pr == 0),
                            stop=(pr == 1),
                        )
                    if pr == 1:
                        for ps, dt in ((st["a"], dtA), (st["b"], dtB)):
                            y_t = small.tile([P, QC], F16, name="y_t", tag="YS")
                            nc.vector.tensor_copy(out=y_t[:], in_=ps[:])
                            nc.gpsimd.dma_start(out_ext[dt, qc], y_t[:])

                return [functools.partial(step, pr) for pr in range(2)]

            def phase1_steps(tc_i):
                return (pair_unit(qk_pair_step, tc_i, 0, 2)
                        + pair_unit(qk_pair_step, tc_i, 1, 3)
                        + pair_unit(v_pair_step, tc_i, 0, 1)
                        + pair_unit(v_pair_step, tc_i, 2, 3))

            # ---- attention ----
            def attn_pair(qc, hp, consume):  # consume() called once per slot
                h0, h1 = 2 * hp, 2 * hp + 1
                qpl, kpl = hp, 2 + hp
                nkt = 4 * qc + 4
                po0 = psO.tile([65, QC], F32, name=f"po_{qc}_{h0}", tag="O")
                po1 = psO.tile([65, QC], F32, name=f"po_{qc}_{h1}", tag="O")
                pending = None

                def emit_av(w_pair, kb, kl, csl, kt):
                    nc.tensor.matmul(
                        po0[:, csl],
                        lhsT=v_t[kb][:, kl, 65 * h0 : 65 * h0 + 65],
                        rhs=w_pair[:, 0, csl],
                        start=(kt == 0),
                        stop=(kt == nkt - 1),
                    )
                    nc.tensor.matmul(
                        po1[:, csl],
                        lhsT=v_t[kb][:, kl, 65 * h1 : 65 * h1 + 65],
                        rhs=w_pair[:, 1, csl],
                        start=(kt == 0),
                        stop=(kt == nkt - 1),
                    )

                for kt in range(nkt):
                    j = kt - 4 * qc
                    c0 = max(0, j) * P
                    csl = slice(c0, QC)
                    kb, kl = kt // 4, kt % 4
                    ps_pair = psS.tile([P, 2, QC], F32,
                                       name=f"ps_s_{qc}_{hp}_{kt}", tag="S")
                    nc.tensor.matmul(
                        ps_pair[:, 0, csl],
                        lhsT=qk_t[kb][0:64, kpl, kl * P : (kl + 1) * P],
                        rhs=qk_t[qc][0:64, qpl, c0:QC],
                        start=True, stop=True,
                    )
                    nc.tensor.matmul(
                        ps_pair[:, 1, csl],
                        lhsT=qk_t[kb][64:128, kpl, kl * P : (kl + 1) * P],
                        rhs=qk_t[qc][64:128, qpl, c0:QC],
                        start=True, stop=True,
                    )
                    if j >= 0:
                        # additive causal mask on PSUM, off the AV dep chain
                        nc.vector.tensor_tensor(
                            out=ps_pair[:, :, c0 : c0 + P],
                            in0=ps_pair[:, :, c0 : c0 + P],
                            in1=mask_sb[:],
                            op=mybir.AluOpType.add,
                        )
                    w_pair = wtpool.tile([P, 2, QC], BF16, name="w_pair", tag="W")
                    nc.scalar.activation(
                        w_pair[:, :, csl], ps_pair[:, :, csl],
                        mybir.ActivationFunctionType.Exp,
                    )
                    if pending is not None:
                        consume()
                        emit_av(*pending)
                    pending = (w_pair, kb, kl, csl, kt)
                emit_av(*pending)
                return po0, po1

            def finish_pair(o_t, hp, po0, po1):
                dens, recs, rbcs = [], [], []
                for po in (po0, po1):
                    den = small.tile([1, QC], F32, name="den", tag="DN")
                    nc.vector.tensor_copy(out=den[:], in_=po[64:65, :])
                    dens.append(den)
                for den in dens:
                    rec = small.tile([1, QC], F32, name="rec", tag="R")
                    nc.vector.reciprocal_approx_fast(out=rec[:], in_=den[:])
                    recs.append(rec)
                for rec in recs:
                    rbc = small.tile([64, QC], F32, name="rbc", tag="RB")
                    nc.gpsimd.partition_broadcast(rbc[:], rec[:])
                    rbcs.append(rbc)
                for po, pb, rbc in ((po0, 0, rbcs[0]), (po1, 64, rbcs[1])):
                    nc.vector.tensor_tensor(
                        out=o_t[pb : pb + 64, hp, :],
                        in0=po[0:64, :],
                        in1=rbc[:],
                        op=mybir.AluOpType.mult,
                    )

            def attn_block(qc, s_hp0, s_rest):
                """s_hp0 drains inside the hp=0 pair (pre-AV slots); s_rest
                spreads across BOTH pairs and drains at block end."""
                o_t = opool.tile([P, 2, QC], F16, name="o_t", tag="OT")
                nkt = 4 * qc + 4
                nslots = 2 * (nkt - 1)
                rest = list(s_rest)
                rdone = [0]
                slot = [0]

                def consume_rest():
                    slot[0] += 1
                    want = (slot[0] * len(rest)) // max(1, nslots)
                    while rdone[0] < min(want, len(rest)):
                        rest[rdone[0]]()
                        rdone[0] += 1

                for hp in range(2):
                    if hp == 0 and s_hp0:
                        h0stream = list(s_hp0)
                        h0done = [0]

                        def consume(kt=None, h0stream=h0stream, h0done=h0done):
                            slot[0] += 1
                            want = (slot[0] * len(h0stream)) // (nkt - 1)
                            while h0done[0] < min(want, len(h0stream)):
                                h0stream[h0done[0]]()
                                h0done[0] += 1
                    else:
                        consume = consume_rest
                    po0, po1 = attn_pair(qc, hp, consume)
                    if hp == 0 and s_hp0:
                        for f in h0stream[h0done[0]:]:
                            f()
                        slot[0] = nkt - 1
                    finish_pair(o_t, hp, po0, po1)
                for f in rest[rdone[0]:]:
                    f()
                return o_t

            # ---- schedule ----
            # prologue: Q/K pair0 of chunk 0 only; V chunk 0 + Q/K pair1
            # stream into block 0's hp0 pre-AV slots
            for f in pair_unit(qk_pair_step, 0, 0, 2):
                f()
            o_prev = None
            for qc in range(NQC):
                if qc == 0:
                    s_hp0 = (pair_unit(v_pair_step, 0, 0, 1)
                             + pair_unit(v_pair_step, 0, 2, 3)
                             + pair_unit(qk_pair_step, 0, 1, 3))
                    s_rest = phase1_steps(1)
                else:
                    s_hp0 = []
                    s_rest = []
                    if qc + 1 < NQC:
                        s_rest += phase1_steps(qc + 1)
                    for dtA in (0, 2, 4, 6):
                        s_rest += cproj_steps(qc - 1, o_prev, dtA, dtA + 1)
                o_prev = attn_block(qc, s_hp0, s_rest)
            for dtA in (0, 2, 4, 6):
                for f in cproj_steps(NQC - 1, o_prev, dtA, dtA + 1):
                    f()

    print(f"[build] traced+scheduled {_time.time()-_t0:.1f}s", flush=True)
    nc.compile()
    print(f"[build] compiled {_time.time()-_t0:.1f}s", flush=True)
    return nc


def _get_nc():
    if "nc" not in _CACHE:
        _CACHE["nc"] = _build()
    return _CACHE["nc"]


def _make_in_maps(x, W_attn, W_proj):
    import ml_dtypes
    Wp = W_proj.reshape(NH, C, HD)  # [head, dout, d]
    A = Wp.reshape(8, 2, C, HD)     # [pair, hl, dout, d]
    wp_all = np.ascontiguousarray(
        A.transpose(1, 3, 0, 2).reshape(P, 8, C)
    ).astype(np.float16)
    m = np.where(np.triu(np.ones([P, P], dtype=np.float32)) > 0,
                 np.float32(0.0), np.float32(-1e30))
    mask_host = np.ascontiguousarray(
        np.broadcast_to(m[:, None, :], (P, 2, P))
    ).astype(np.float32)

    def part_major(a):  # [KSUB*P, M] -> [P, KSUB, M]
        return np.ascontiguousarray(
            a.reshape(KSUB, P, a.shape[1]).transpose(1, 0, 2)
        )

    in_maps = []
    for core in range(NCORE):
        b, g = core // 4, core % 4
        xt = x[b].T.astype(np.float16)  # [C, T]
        # [NQC, P, KSUB, QC]
        xt4 = np.ascontiguousarray(
            xt.reshape(KSUB, P, NQC, QC).transpose(2, 1, 0, 3)
        )
        Wq = W_attn[256 * g : 256 * (g + 1)]
        Wk = W_attn[C + 256 * g : C + 256 * (g + 1)]
        Wv = W_attn[2 * C + 256 * g : 2 * C + 256 * (g + 1)]
        wqk = part_major(
            np.concatenate([Wq, Wk], 0).T.astype(np.float16))  # [P,KSUB,512]
        wv = part_major(Wv.T.astype(np.float16))                # [P,KSUB,256]
        wp = np.ascontiguousarray(wp_all[:, 2 * g : 2 * g + 2, :])
        in_maps.append(
            {"xt": xt4, "wqk": wqk, "wv": wv, "wp": wp, "mask": mask_host}
        )
    return in_maps


def kernel(x, W_attn, W_proj):
    from concourse.bass_utils import run_bass_kernel_spmd

    x = np.asarray(x, dtype=np.float32)
    W_attn = np.asarray(W_attn, dtype=np.float32)
    W_proj = np.asarray(W_proj, dtype=np.float32)

    in_maps = _make_in_maps(x, W_attn, W_proj)
    nc = _get_nc()
    res = run_bass_kernel_spmd(nc, in_maps, core_ids=list(range(NCORE)))
    _CACHE["last_result"] = res

    Y = np.empty((B, T, C), dtype=np.float32)
    for b in range(B):
        acc = np.zeros((C, T), dtype=np.float32)
        for g in range(4):
            o = np.asarray(res.results[4 * b + g]["out"], dtype=np.float32)
            acc += o.transpose(0, 2, 1, 3).reshape(C, T)
        Y[b] = acc.T
    return Y


# revision 9
# speedup vs baseline: 1.1304x; 1.1304x over previous
"""Trainium2 8-core Bass kernel for nn_AttentionHPROJ (B=2,T=2048,C=1024,h=16,hd=64).

Sharding: core = 4*b + g owns batch b and heads [4g, 4g+4).
All compute in transposed layout (dout on partitions, tokens moving).

v3: fp16/bf16 matmul dataflow (PSUM accumulation fp32):
  - Q^T/K^T = wqk^T @ x^T in fp16; V natural = x @ Wv^T -> bf16.
  - S^T per head pair packed in one [128,2,512] PSUM tile; h-even on PE rows
    0:64, h-odd on rows 64:128 (concurrent quadrant execution, ~2x).
  - One exp (ACT) per head PAIR over [128,2,width] -> w_pair bf16; causal
    mask multiply on diagonal subtiles (DVE).
  - AV: O^T per head in [65,512] PSUM, ones-column softmax denominator.
  - Normalize: copy denom row to SBUF, reciprocal_approx_fast (the custom op
    NaNs when reading PSUM directly), gpsimd partition_broadcast, DVE mult.
  - c_proj fp16 -> partial Y^T per core, fp16 out; host sums quad + transposes.
  - All proj/c_proj matmul chains run as interleaved PAIRS (alternating PSUM
    banks) so back-to-back matmuls pipeline (~218ns vs ~430ns each).
  - Inputs host-relayouted to SBUF layout: every DMA is [128 part x contiguous
    bytes]; bulk tensors land in 1-2 DMAs; y-out DMAs dispatched from gpsimd.
"""
import sys

sys.path.insert(0, "/opt/trn_rl_repo")

import numpy as np

B, T, C = 2, 2048, 1024
NH, HD = 16, 64
P = 128
NCORE = 8
QC = 512          # q-chunk width
NQC = T // QC     # 4
KSUB = C // P     # 8

_CACHE = {}


def _build():
    import functools
    import concourse.bass as bass
    import concourse.mybir as mybir
    import concourse.tile as tile
    from concourse import bacc

    F32 = mybir.dt.float32
    F32R = mybir.dt.float32r
    F16 = mybir.dt.float16
    BF16 = mybir.dt.bfloat16

    import time as _time
    _t0 = _time.time()
    print("[build] start v3", flush=True)
    nc = bacc.Bacc("TRN2", target_bir_lowering=False, debug=False, num_devices=NCORE)

    xt_ext = nc.dram_tensor("xt", [NQC, P, KSUB, QC], F32R, kind="ExternalInput").ap()
    wqk_ext = nc.dram_tensor("wqk", [P, KSUB, 512], F32R, kind="ExternalInput").ap()
    wv_ext = nc.dram_tensor("wv", [P, KSUB, 256], F32R, kind="ExternalInput").ap()
    wp_ext = nc.dram_tensor("wp", [P, 2, C], BF16, kind="ExternalInput").ap()
    mask_ext = nc.dram_tensor("mask", [P, 2, P], BF16, kind="ExternalInput").ap()
    out_ext = nc.dram_tensor("out", [KSUB, NQC, P, QC], F16,
                             kind="ExternalOutput").ap()

    with tile.TileContext(nc) as tc:
        with (
            tc.tile_pool(name="wpool", bufs=1) as wpool,
            tc.tile_pool(name="xpool", bufs=1) as xpool,
            tc.tile_pool(name="qkpool", bufs=1) as qkpool,
            tc.tile_pool(name="vpool", bufs=1) as vpool,
            tc.tile_pool(name="wtile", bufs=4) as wtpool,
            tc.tile_pool(name="opool", bufs=2) as opool,
            tc.tile_pool(name="small", bufs=4) as small,
            tc.tile_pool(name="psS", bufs=2, space="PSUM") as psS,
            tc.tile_pool(name="psO", bufs=2, space="PSUM") as psO,
            tc.tile_pool(name="psY", bufs=2, space="PSUM") as psY,
        ):
            # ---- inputs ----
            wqk_sb = wpool.tile([P, KSUB, 512], F32R)
            wv_sb = wpool.tile([P, KSUB, 256], F32R)
            xt_t = [xpool.tile([P, KSUB, QC], F32R, name=f"xt_{i}", tag=f"xt_{i}")
                    for i in range(NQC)]
            qk_t = [qkpool.tile([P, 4, QC], F32R, name=f"qk_{i}", tag=f"qk_{i}")
                    for i in range(NQC)]
            v_t = [vpool.tile([P, 4, 4 * 65], BF16, name=f"v_{i}", tag=f"v_{i}")
                   for i in range(NQC)]
            mask_sb = wpool.tile([P, 2, P], BF16)
            wp_sb = wpool.tile([P, 2, C], BF16)

            nc.sync.dma_start(mask_sb[:], mask_ext[:])
            ones_st = wpool.tile([P, 4, 4], BF16)
            nc.vector.memset(ones_st[:], 1.0)
            for i in range(NQC):
                ov = v_t[i].rearrange("p t (h c) -> p t h c", c=65)[:, :, :, 64]
                nc.vector.tensor_copy(out=ov, in_=ones_st[:])
            # early-need inputs on sync, halves for low first-matmul latency
            nc.sync.dma_start(wqk_sb[:, 0:2], wqk_ext[:, 0:2])
            nc.sync.dma_start(xt_t[0][:, 0:4], xt_ext[0, :, 0:4])
            nc.sync.dma_start(wqk_sb[:, 2:4], wqk_ext[:, 2:4])
            nc.sync.dma_start(wv_sb[:], wv_ext[:])
            nc.sync.dma_start(wqk_sb[:, 4:8], wqk_ext[:, 4:8])
            nc.sync.dma_start(xt_t[0][:, 4:8], xt_ext[0, :, 4:8])
            nc.sync.dma_start(wp_sb[:], wp_ext[:])
            for i in range(1, NQC):
                nc.sync.dma_start(xt_t[i][:], xt_ext[i])

            # ---- interleaved-pair projection chains ----
            def qk_pair_step(tc_i, dtA, dtB, st, k):
                if k == 0:
                    st["a"] = psY.tile([P, QC], F32, name=f"pqk{tc_i}{dtA}", tag="Y")
                    st["b"] = psY.tile([P, QC], F32, name=f"pqk{tc_i}{dtB}", tag="Y")
                for ps, dt in ((st["a"], dtA), (st["b"], dtB)):
                    nc.tensor.matmul(
                        ps[:],
                        lhsT=wqk_sb[:, k, dt * P : (dt + 1) * P],
                        rhs=xt_t[tc_i][:, k],
                        start=(k == 0),
                        stop=(k == KSUB - 1),
                    )
                if k == KSUB - 1:
                    nc.vector.tensor_copy(out=qk_t[tc_i][:, dtA, :], in_=st["a"][:])
                    nc.vector.tensor_copy(out=qk_t[tc_i][:, dtB, :], in_=st["b"][:])

            def v_pair_step(tc_i, tlA, tlB, st, k):
                if k == 0:
                    st["a"] = psY.tile([P, 256], F32, name=f"pv{tc_i}{tlA}", tag="Y")
                    st["b"] = psY.tile([P, 256], F32, name=f"pv{tc_i}{tlB}", tag="Y")
                for ps, tl_ in ((st["a"], tlA), (st["b"], tlB)):
                    nc.tensor.matmul(
                        ps[:],
                        lhsT=xt_t[tc_i][:, k, tl_ * P : (tl_ + 1) * P],
                        rhs=wv_sb[:, k, :],
                        start=(k == 0),
                        stop=(k == KSUB - 1),
                    )
                if k == KSUB - 1:
                    for ps, tl_ in ((st["a"], tlA), (st["b"], tlB)):
                        vdst = v_t[tc_i].rearrange(
                            "p t (h c) -> p t h c", c=65)[:, tl_, :, 0:64]
                        nc.vector.tensor_copy(
                            out=vdst, in_=ps.rearrange("p (h d) -> p h d", d=64)
                        )

            def pair_unit(fn, *args):
                st = {}
                return [functools.partial(fn, *args, st, k) for k in range(KSUB)]

            def cproj_steps(qc, o_t, dtA, dtB):
                st = {}

                def step(pr):
                    if pr == 0:
                        st["a"] = psY.tile([P, QC], F32, name=f"py{qc}{dtA}", tag="Y")
                        st["b"] = psY.tile([P, QC], F32, name=f"py{qc}{dtB}", tag="Y")
                    for ps, dt in ((st["a"], dtA), (st["b"], dtB)):
                        nc.tensor.matmul(
                            ps[:],
                            lhsT=wp_sb[:, pr, dt * P : (dt + 1) * P],
                            rhs=o_t[:, pr, :],
                            start=(pr == 0),
                            stop=(pr == 1),
                        )
                    if pr == 1:
                        for ps, dt in ((st["a"], dtA), (st["b"], dtB)):
                            y_t = small.tile([P, QC], F16, name="y_t", tag="YS")
                            nc.vector.tensor_copy(out=y_t[:], in_=ps[:])
                            nc.gpsimd.dma_start(out_ext[dt, qc], y_t[:])

                return [functools.partial(step, pr) for pr in range(2)]

            def phase1_steps(tc_i):
                return (pair_unit(qk_pair_step, tc_i, 0, 2)
                        + pair_unit(qk_pair_step, tc_i, 1, 3)
                        + pair_unit(v_pair_step, tc_i, 0, 1)
                        + pair_unit(v_pair_step, tc_i, 2, 3))

            # ---- attention ----
            def attn_pair(qc, hp, consume):  # consume() called once per slot
                h0, h1 = 2 * hp, 2 * hp + 1
                qpl, kpl = hp, 2 + hp
                nkt = 4 * qc + 4
                po0 = psO.tile([65, QC], F32, name=f"po_{qc}_{h0}", tag="O")
                po1 = psO.tile([65, QC], F32, name=f"po_{qc}_{h1}", tag="O")
                pending = None

                def emit_av(w_pair, kb, kl, csl, kt):
                    nc.tensor.matmul(
                        po0[:, csl],
                        lhsT=v_t[kb][:, kl, 65 * h0 : 65 * h0 + 65],
                        rhs=w_pair[:, 0, csl],
                        start=(kt == 0),
                        stop=(kt == nkt - 1),
                    )
                    nc.tensor.matmul(
                        po1[:, csl],
                        lhsT=v_t[kb][:, kl, 65 * h1 : 65 * h1 + 65],
                        rhs=w_pair[:, 1, csl],
                        start=(kt == 0),
                        stop=(kt == nkt - 1),
                    )

                for kt in range(nkt):
                    j = kt - 4 * qc
                    c0 = max(0, j) * P
                    csl = slice(c0, QC)
                    kb, kl = kt // 4, kt % 4
                    ps_pair = psS.tile([P, 2, QC], F32,
                                       name=f"ps_s_{qc}_{hp}_{kt}", tag="S")
                    nc.tensor.matmul(
                        ps_pair[:, 0, csl],
                        lhsT=qk_t[kb][0:64, kpl, kl * P : (kl + 1) * P],
                        rhs=qk_t[qc][0:64, qpl, c0:QC],
                        start=True, stop=True,
                    )
                    nc.tensor.matmul(
                        ps_pair[:, 1, csl],
                        lhsT=qk_t[kb][64:128, kpl, kl * P : (kl + 1) * P],
                        rhs=qk_t[qc][64:128, qpl, c0:QC],
                        start=True, stop=True,
                    )
                    w_pair = wtpool.tile([P, 2, QC], BF16, name="w_pair", tag="W")
                    nc.scalar.activation(
                        w_pair[:, :, csl], ps_pair[:, :, csl],
                        mybir.ActivationFunctionType.Exp,
                    )
                    if j >= 0:
                        nc.vector.tensor_tensor(
                            out=w_pair[:, :, c0 : c0 + P],
                            in0=w_pair[:, :, c0 : c0 + P],
                            in1=mask_sb[:],
                            op=mybir.AluOpType.mult,
                        )
                    if pending is not None:
                        consume()
                        emit_av(*pending)
                    pending = (w_pair, kb, kl, csl, kt)
                emit_av(*pending)
                return po0, po1

            def finish_pair(o_t, hp, po0, po1):
                dens, recs, rbcs = [], [], []
                for po in (po0, po1):
                    den = small.tile([1, QC], F32, name="den", tag="DN")
                    nc.vector.tensor_copy(out=den[:], in_=po[64:65, :])
                    dens.append(den)
                for den in dens:
                    rec = small.tile([1, QC], F32, name="rec", tag="R")
                    nc.vector.reciprocal_approx_fast(out=rec[:], in_=den[:])
                    recs.append(rec)
                for rec in recs:
                    rbc = small.tile([64, QC], F32, name="rbc", tag="RB")
                    nc.gpsimd.partition_broadcast(rbc[:], rec[:])
                    rbcs.append(rbc)
                for po, pb, rbc in ((po0, 0, rbcs[0]), (po1, 64, rbcs[1])):
                    nc.vector.tensor_tensor(
                        out=o_t[pb : pb + 64, hp, :],
                        in0=po[0:64, :],
                        in1=rbc[:],
                        op=mybir.AluOpType.mult,
                    )

            def attn_block(qc, s_hp0, s_rest):
                """s_hp0 drains inside the hp=0 pair (pre-AV slots); s_rest
                spreads across BOTH pairs and drains at block end."""
                o_t = opool.tile([P, 2, QC], BF16, name="o_t", tag="OT")
                nkt = 4 * qc + 4
                nslots = 2 * (nkt - 1)
                rest = list(s_rest)
                rdone = [0]
                slot = [0]

                def consume_rest():
                    slot[0] += 1
                    want = (slot[0] * len(rest)) // max(1, nslots)
                    while rdone[0] < min(want, len(rest)):
                        rest[rdone[0]]()
                        rdone[0] += 1

                for hp in range(2):
                    if hp == 0 and s_hp0:
                        h0stream = list(s_hp0)
                        h0done = [0]

                        def consume(kt=None, h0stream=h0stream, h0done=h0done):
                            slot[0] += 1
                            want = (slot[0] * len(h0stream)) // (nkt - 1)
                            while h0done[0] < min(want, len(h0stream)):
                                h0stream[h0done[0]]()
                                h0done[0] += 1
                    else:
                        consume = consume_rest
                    po0, po1 = attn_pair(qc, hp, consume)
                    if hp == 0 and s_hp0:
                        for f in h0stream[h0done[0]:]:
                            f()
                        slot[0] = nkt - 1
                    finish_pair(o_t, hp, po0, po1)
                for f in rest[rdone[0]:]:
                    f()
                return o_t

            # ---- schedule ----
            # prologue: Q/K pair0 of chunk 0 only; V chunk 0 + Q/K pair1
            # stream into block 0's hp0 pre-AV slots
            for f in pair_unit(qk_pair_step, 0, 0, 2):
                f()
            o_prev = None
            for qc in range(NQC):
                if qc == 0:
                    s_hp0 = (pair_unit(v_pair_step, 0, 0, 1)
                             + pair_unit(v_pair_step, 0, 2, 3)
                             + pair_unit(qk_pair_step, 0, 1, 3))
                    s_rest = phase1_steps(1)
                else:
                    s_hp0 = []
                    s_rest = []
                    if qc + 1 < NQC:
                        s_rest += phase1_steps(qc + 1)
                    for dtA in (0, 2, 4, 6):
                        s_rest += cproj_steps(qc - 1, o_prev, dtA, dtA + 1)
                o_prev = attn_block(qc, s_hp0, s_rest)
            for dtA in (0, 2, 4, 6):
                for f in cproj_steps(NQC - 1, o_prev, dtA, dtA + 1):
                    f()

    print(f"[build] traced+scheduled {_time.time()-_t0:.1f}s", flush=True)
    nc.compile()
    print(f"[build] compiled {_time.time()-_t0:.1f}s", flush=True)
    return nc


def _get_nc():
    if "nc" not in _CACHE:
        _CACHE["nc"] = _build()
    return _CACHE["nc"]


def _make_in_maps(x, W_attn, W_proj):
    import ml_dtypes
    Wp = W_proj.reshape(NH, C, HD)  # [head, dout, d]
    A = Wp.reshape(8, 2, C, HD)     # [pair, hl, dout, d]
    wp_all = np.ascontiguousarray(
        A.transpose(1, 3, 0, 2).reshape(P, 8, C)
    ).astype(ml_dtypes.bfloat16)
    m = np.triu(np.ones([P, P], dtype=np.float32))
    mask_host = np.ascontiguousarray(
        np.broadcast_to(m[:, None, :], (P, 2, P))
    ).astype(ml_dtypes.bfloat16)

    def part_major(a):  # [KSUB*P, M] -> [P, KSUB, M]
        return np.ascontiguousarray(
            a.reshape(KSUB, P, a.shape[1]).transpose(1, 0, 2)
        )

    in_maps = []
    for core in range(NCORE):
        b, g = core // 4, core % 4
        xt = np.ascontiguousarray(x[b].T)  # [C, T] fp32
        # [NQC, P, KSUB, QC]
        xt4 = np.ascontiguousarray(
            xt.reshape(KSUB, P, NQC, QC).transpose(2, 1, 0, 3)
        )
        Wq = W_attn[256 * g : 256 * (g + 1)]
        Wk = W_attn[C + 256 * g : C + 256 * (g + 1)]
        Wv = W_attn[2 * C + 256 * g : 2 * C + 256 * (g + 1)]
        wqk = part_major(np.ascontiguousarray(
            np.concatenate([Wq, Wk], 0).T))         # [P,KSUB,512] fp32
        wv = part_major(np.ascontiguousarray(Wv.T))  # [P,KSUB,256] fp32
        wp = np.ascontiguousarray(wp_all[:, 2 * g : 2 * g + 2, :])
        in_maps.append(
            {"xt": xt4, "wqk": wqk, "wv": wv, "wp": wp, "mask": mask_host}
        )
    return in_maps


def kernel(x, W_attn, W_proj):
    from concourse.bass_utils import run_bass_kernel_spmd

    x = np.asarray(x, dtype=np.float32)
    W_attn = np.asarray(W_attn, dtype=np.float32)
    W_proj = np.asarray(W_proj, dtype=np.float32)

    in_maps = _make_in_maps(x, W_attn, W_proj)
    nc = _get_nc()
    res = run_bass_kernel_spmd(nc, in_maps, core_ids=list(range(NCORE)))
    _CACHE["last_result"] = res

    Y = np.empty((B, T, C), dtype=np.float32)
    for b in range(B):
        acc = np.zeros((C, T), dtype=np.float32)
        for g in range(4):
            o = np.asarray(res.results[4 * b + g]["out"], dtype=np.float32)
            acc += o.transpose(0, 2, 1, 3).reshape(C, T)
        Y[b] = acc.T
    return Y


# revision 11
# speedup vs baseline: 1.1578x; 1.0242x over previous
"""Trainium2 8-core Bass kernel for nn_AttentionHPROJ (B=2,T=2048,C=1024,h=16,hd=64).

Sharding: core = 4*b + g owns batch b and heads [4g, 4g+4).
All compute in transposed layout (dout on partitions, tokens moving).

v3: fp16/bf16 matmul dataflow (PSUM accumulation fp32):
  - Q^T/K^T = wqk^T @ x^T in fp16; V natural = x @ Wv^T -> bf16.
  - S^T per head pair packed in one [128,2,512] PSUM tile; h-even on PE rows
    0:64, h-odd on rows 64:128 (concurrent quadrant execution, ~2x).
  - One exp (ACT) per head PAIR over [128,2,width] -> w_pair bf16; causal
    mask multiply on diagonal subtiles (DVE).
  - AV: O^T per head in [65,512] PSUM, ones-column softmax denominator.
  - Normalize: copy denom row to SBUF, reciprocal_approx_fast (the custom op
    NaNs when reading PSUM directly), gpsimd partition_broadcast, DVE mult.
  - c_proj fp16 -> partial Y^T per core, fp16 out; host sums quad + transposes.
  - All proj/c_proj matmul chains run as interleaved PAIRS (alternating PSUM
    banks) so back-to-back matmuls pipeline (~218ns vs ~430ns each).
  - Inputs host-relayouted to SBUF layout: every DMA is [128 part x contiguous
    bytes]; bulk tensors land in 1-2 DMAs; y-out DMAs dispatched from gpsimd.
"""
import sys

sys.path.insert(0, "/opt/trn_rl_repo")

import numpy as np

B, T, C = 2, 2048, 1024
NH, HD = 16, 64
P = 128
NCORE = 8
QC = 512          # q-chunk width
NQC = T // QC     # 4
KSUB = C // P     # 8

_CACHE = {}


def _build():
    import functools
    import concourse.bass as bass
    import concourse.mybir as mybir
    import concourse.tile as tile
    from concourse import bacc

    F32 = mybir.dt.float32
    F32R = mybir.dt.float32r
    F16 = mybir.dt.float16
    BF16 = mybir.dt.bfloat16

    import time as _time
    _t0 = _time.time()
    print("[build] start v3", flush=True)
    nc = bacc.Bacc("TRN2", target_bir_lowering=False, debug=False, num_devices=NCORE)

    xt_ext = nc.dram_tensor("xt", [NQC, P, KSUB, QC], F16, kind="ExternalInput").ap()
    wqk_ext = nc.dram_tensor("wqk", [P, KSUB, 512], F16, kind="ExternalInput").ap()
    wv_ext = nc.dram_tensor("wv", [P, KSUB, 256], F16, kind="ExternalInput").ap()
    wp_ext = nc.dram_tensor("wp", [P, 2, C], F16, kind="ExternalInput").ap()
    mask_ext = nc.dram_tensor("mask", [P, 2, P], BF16, kind="ExternalInput").ap()
    out_ext = nc.dram_tensor("out", [KSUB, NQC, P, QC], F16,
                             kind="ExternalOutput").ap()

    with tile.TileContext(nc) as tc:
        with (
            tc.tile_pool(name="wpool", bufs=1) as wpool,
            tc.tile_pool(name="xpool", bufs=1) as xpool,
            tc.tile_pool(name="qkpool", bufs=1) as qkpool,
            tc.tile_pool(name="vpool", bufs=1) as vpool,
            tc.tile_pool(name="wtile", bufs=4) as wtpool,
            tc.tile_pool(name="opool", bufs=2) as opool,
            tc.tile_pool(name="small", bufs=4) as small,
            tc.tile_pool(name="psS", bufs=2, space="PSUM") as psS,
            tc.tile_pool(name="psO", bufs=2, space="PSUM") as psO,
            tc.tile_pool(name="psY", bufs=2, space="PSUM") as psY,
        ):
            # ---- inputs ----
            wqk_sb = wpool.tile([P, KSUB, 512], F16)
            wv_sb = wpool.tile([P, KSUB, 256], F16)
            xt_t = [xpool.tile([P, KSUB, QC], F16, name=f"xt_{i}", tag=f"xt_{i}")
                    for i in range(NQC)]
            qk_t = [qkpool.tile([P, 4, QC], F16, name=f"qk_{i}", tag=f"qk_{i}")
                    for i in range(NQC)]
            v_t = [vpool.tile([P, 4, 4 * 65], BF16, name=f"v_{i}", tag=f"v_{i}")
                   for i in range(NQC)]
            mask_sb = wpool.tile([P, 2, P], BF16)
            wp_sb = wpool.tile([P, 2, C], F16)

            nc.sync.dma_start(mask_sb[:], mask_ext[:])
            ones_st = wpool.tile([P, 4, 4], BF16)
            nc.vector.memset(ones_st[:], 1.0)
            for i in range(NQC):
                ov = v_t[i].rearrange("p t (h c) -> p t h c", c=65)[:, :, :, 64]
                nc.vector.tensor_copy(out=ov, in_=ones_st[:])
            # early-need inputs on sync, halves for low first-matmul latency
            nc.sync.dma_start(wqk_sb[:, 0:4], wqk_ext[:, 0:4])
            nc.gpsimd.dma_start(xt_t[0][:, 0:4], xt_ext[0, :, 0:4])
            nc.sync.dma_start(wqk_sb[:, 4:8], wqk_ext[:, 4:8])
            nc.gpsimd.dma_start(xt_t[0][:, 4:8], xt_ext[0, :, 4:8])
            nc.scalar.dma_start(wv_sb[:], wv_ext[:])
            nc.scalar.dma_start(wp_sb[:], wp_ext[:])
            nc.gpsimd.dma_start(xt_t[1][:], xt_ext[1])
            nc.scalar.dma_start(xt_t[2][:], xt_ext[2])
            nc.gpsimd.dma_start(xt_t[3][:], xt_ext[3])

            # ---- interleaved-pair projection chains ----
            def qk_pair_step(tc_i, dtA, dtB, st, k):
                if k == 0:
                    st["a"] = psY.tile([P, QC], F32, name=f"pqk{tc_i}{dtA}", tag="Y")
                    st["b"] = psY.tile([P, QC], F32, name=f"pqk{tc_i}{dtB}", tag="Y")
                for ps, dt in ((st["a"], dtA), (st["b"], dtB)):
                    nc.tensor.matmul(
                        ps[:],
                        lhsT=wqk_sb[:, k, dt * P : (dt + 1) * P],
                        rhs=xt_t[tc_i][:, k],
                        start=(k == 0),
                        stop=(k == KSUB - 1),
                    )
                if k == KSUB - 1:
                    nc.vector.tensor_copy(out=qk_t[tc_i][:, dtA, :], in_=st["a"][:])
                    nc.vector.tensor_copy(out=qk_t[tc_i][:, dtB, :], in_=st["b"][:])

            def v_pair_step(tc_i, tlA, tlB, st, k):
                if k == 0:
                    st["a"] = psY.tile([P, 256], F32, name=f"pv{tc_i}{tlA}", tag="Y")
                    st["b"] = psY.tile([P, 256], F32, name=f"pv{tc_i}{tlB}", tag="Y")
                for ps, tl_ in ((st["a"], tlA), (st["b"], tlB)):
                    nc.tensor.matmul(
                        ps[:],
                        lhsT=xt_t[tc_i][:, k, tl_ * P : (tl_ + 1) * P],
                        rhs=wv_sb[:, k, :],
                        start=(k == 0),
                        stop=(k == KSUB - 1),
                    )
                if k == KSUB - 1:
                    for ps, tl_ in ((st["a"], tlA), (st["b"], tlB)):
                        vdst = v_t[tc_i].rearrange(
                            "p t (h c) -> p t h c", c=65)[:, tl_, :, 0:64]
                        nc.vector.tensor_copy(
                            out=vdst, in_=ps.rearrange("p (h d) -> p h d", d=64)
                        )

            def pair_unit(fn, *args):
                st = {}
                return [functools.partial(fn, *args, st, k) for k in range(KSUB)]

            def cproj_steps(qc, o_t, dtA, dtB, use_act=False):
                st = {}

                def step(pr):
                    if pr == 0:
                        st["a"] = psY.tile([P, QC], F32, name=f"py{qc}{dtA}", tag="Y")
                        st["b"] = psY.tile([P, QC], F32, name=f"py{qc}{dtB}", tag="Y")
                    for ps, dt in ((st["a"], dtA), (st["b"], dtB)):
                        nc.tensor.matmul(
                            ps[:],
                            lhsT=wp_sb[:, pr, dt * P : (dt + 1) * P],
                            rhs=o_t[:, pr, :],
                            start=(pr == 0),
                            stop=(pr == 1),
                        )
                    if pr == 1:
                        for i, (ps, dt) in enumerate(((st["a"], dtA), (st["b"], dtB))):
                            y_t = small.tile([P, QC], F16, name="y_t", tag="YS")
                            if use_act and i == 0:
                                nc.scalar.copy(out=y_t[:], in_=ps[:])
                            else:
                                nc.vector.tensor_copy(out=y_t[:], in_=ps[:])
                            nc.gpsimd.dma_start(out_ext[dt, qc], y_t[:])

                return [functools.partial(step, pr) for pr in range(2)]

            def phase1_steps(tc_i):
                return (pair_unit(qk_pair_step, tc_i, 0, 2)
                        + pair_unit(qk_pair_step, tc_i, 1, 3)
                        + pair_unit(v_pair_step, tc_i, 0, 1)
                        + pair_unit(v_pair_step, tc_i, 2, 3))

            # ---- attention ----
            def attn_pair(qc, hp, consume):  # consume() called once per slot
                h0, h1 = 2 * hp, 2 * hp + 1
                qpl, kpl = hp, 2 + hp
                nkt = 4 * qc + 4
                po0 = psO.tile([65, QC], F32, name=f"po_{qc}_{h0}", tag="O")
                po1 = psO.tile([65, QC], F32, name=f"po_{qc}_{h1}", tag="O")
                pending = None

                def emit_av(w_pair, kb, kl, csl, kt):
                    nc.tensor.matmul(
                        po0[:, csl],
                        lhsT=v_t[kb][:, kl, 65 * h0 : 65 * h0 + 65],
                        rhs=w_pair[:, 0, csl],
                        start=(kt == 0),
                        stop=(kt == nkt - 1),
                    )
                    nc.tensor.matmul(
                        po1[:, csl],
                        lhsT=v_t[kb][:, kl, 65 * h1 : 65 * h1 + 65],
                        rhs=w_pair[:, 1, csl],
                        start=(kt == 0),
                        stop=(kt == nkt - 1),
                    )

                for kt in range(nkt):
                    j = kt - 4 * qc
                    c0 = max(0, j) * P
                    csl = slice(c0, QC)
                    kb, kl = kt // 4, kt % 4
                    ps_pair = psS.tile([P, 2, QC], F32,
                                       name=f"ps_s_{qc}_{hp}_{kt}", tag="S")
                    nc.tensor.matmul(
                        ps_pair[:, 0, csl],
                        lhsT=qk_t[kb][0:64, kpl, kl * P : (kl + 1) * P],
                        rhs=qk_t[qc][0:64, qpl, c0:QC],
                        start=True, stop=True,
                    )
                    nc.tensor.matmul(
                        ps_pair[:, 1, csl],
                        lhsT=qk_t[kb][64:128, kpl, kl * P : (kl + 1) * P],
                        rhs=qk_t[qc][64:128, qpl, c0:QC],
                        start=True, stop=True,
                    )
                    w_pair = wtpool.tile([P, 2, QC], BF16, name="w_pair", tag="W")
                    nc.scalar.activation(
                        w_pair[:, :, csl], ps_pair[:, :, csl],
                        mybir.ActivationFunctionType.Exp,
                    )
                    if j >= 0:
                        nc.vector.tensor_tensor(
                            out=w_pair[:, :, c0 : c0 + P],
                            in0=w_pair[:, :, c0 : c0 + P],
                            in1=mask_sb[:],
                            op=mybir.AluOpType.mult,
                        )
                    if pending is not None:
                        consume()
                        emit_av(*pending)
                    pending = (w_pair, kb, kl, csl, kt)
                emit_av(*pending)
                return po0, po1

            def finish_pair(o_t, hp, po0, po1):
                dens, recs, rbcs = [], [], []
                for po in (po0, po1):
                    den = small.tile([1, QC], F32, name="den", tag="DN")
                    nc.vector.tensor_copy(out=den[:], in_=po[64:65, :])
                    dens.append(den)
                for den in dens:
                    rec = small.tile([1, QC], F32, name="rec", tag="R")
                    nc.vector.reciprocal_approx_fast(out=rec[:], in_=den[:])
                    recs.append(rec)
                for rec in recs:
                    rbc = small.tile([64, QC], F32, name="rbc", tag="RB")
                    nc.gpsimd.partition_broadcast(rbc[:], rec[:])
                    rbcs.append(rbc)
                for po, pb, rbc in ((po0, 0, rbcs[0]), (po1, 64, rbcs[1])):
                    nc.vector.tensor_tensor(
                        out=o_t[pb : pb + 64, hp, :],
                        in0=po[0:64, :],
                        in1=rbc[:],
                        op=mybir.AluOpType.mult,
                    )

            def attn_block(qc, s_hp0, s_rest):
                """s_hp0 drains inside the hp=0 pair (pre-AV slots); s_rest
                spreads across BOTH pairs and drains at block end."""
                o_t = opool.tile([P, 2, QC], F16, name="o_t", tag="OT")
                nkt = 4 * qc + 4
                nslots = 2 * (nkt - 1)
                rest = list(s_rest)
                rdone = [0]
                slot = [0]

                def consume_rest():
                    slot[0] += 1
                    want = (slot[0] * len(rest)) // max(1, nslots)
                    while rdone[0] < min(want, len(rest)):
                        rest[rdone[0]]()
                        rdone[0] += 1

                for hp in range(2):
                    if hp == 0 and s_hp0:
                        h0stream = list(s_hp0)
                        h0done = [0]

                        def consume(kt=None, h0stream=h0stream, h0done=h0done):
                            slot[0] += 1
                            want = (slot[0] * len(h0stream)) // (nkt - 1)
                            while h0done[0] < min(want, len(h0stream)):
                                h0stream[h0done[0]]()
                                h0done[0] += 1
                    else:
                        consume = consume_rest
                    po0, po1 = attn_pair(qc, hp, consume)
                    if hp == 0 and s_hp0:
                        for f in h0stream[h0done[0]:]:
                            f()
                        slot[0] = nkt - 1
                    finish_pair(o_t, hp, po0, po1)
                for f in rest[rdone[0]:]:
                    f()
                return o_t

            # ---- schedule ----
            # prologue: Q/K pair0 of chunk 0 only; V chunk 0 + Q/K pair1
            # stream into block 0's hp0 pre-AV slots
            for f in pair_unit(qk_pair_step, 0, 0, 2):
                f()
            o_prev = None
            for qc in range(NQC):
                if qc == 0:
                    s_hp0 = (pair_unit(v_pair_step, 0, 0, 1)
                             + pair_unit(v_pair_step, 0, 2, 3)
                             + pair_unit(qk_pair_step, 0, 1, 3))
                    s_rest = phase1_steps(1)
                else:
                    s_hp0 = []
                    s_rest = []
                    if qc + 1 < NQC:
                        s_rest += phase1_steps(qc + 1)
                    for dtA in (0, 2, 4, 6):
                        s_rest += cproj_steps(qc - 1, o_prev, dtA, dtA + 1)
                o_prev = attn_block(qc, s_hp0, s_rest)
            for dtA in (0, 2, 4, 6):
                for f in cproj_steps(NQC - 1, o_prev, dtA, dtA + 1, use_act=True):
                    f()

    print(f"[build] traced+scheduled {_time.time()-_t0:.1f}s", flush=True)
    nc.compile()
    print(f"[build] compiled {_time.time()-_t0:.1f}s", flush=True)
    return nc


def _get_nc():
    if "nc" not in _CACHE:
        _CACHE["nc"] = _build()
    return _CACHE["nc"]


def _make_in_maps(x, W_attn, W_proj):
    import ml_dtypes
    Wp = W_proj.reshape(NH, C, HD)  # [head, dout, d]
    A = Wp.reshape(8, 2, C, HD)     # [pair, hl, dout, d]
    wp_all = np.ascontiguousarray(
        A.transpose(1, 3, 0, 2).reshape(P, 8, C)
    ).astype(np.float16)
    m = np.triu(np.ones([P, P], dtype=np.float32))
    mask_host = np.ascontiguousarray(
        np.broadcast_to(m[:, None, :], (P, 2, P))
    ).astype(ml_dtypes.bfloat16)

    def part_major(a):  # [KSUB*P, M] -> [P, KSUB, M]
        return np.ascontiguousarray(
            a.reshape(KSUB, P, a.shape[1]).transpose(1, 0, 2)
        )

    in_maps = []
    for core in range(NCORE):
        b, g = core // 4, core % 4
        xt = x[b].T.astype(np.float16)  # [C, T]
        # [NQC, P, KSUB, QC]
        xt4 = np.ascontiguousarray(
            xt.reshape(KSUB, P, NQC, QC).transpose(2, 1, 0, 3)
        )
        Wq = W_attn[256 * g : 256 * (g + 1)]
        Wk = W_attn[C + 256 * g : C + 256 * (g + 1)]
        Wv = W_attn[2 * C + 256 * g : 2 * C + 256 * (g + 1)]
        wqk = part_major(
            np.concatenate([Wq, Wk], 0).T.astype(np.float16))  # [P,KSUB,512]
        wv = part_major(Wv.T.astype(np.float16))               # [P,KSUB,256]
        wp = np.ascontiguousarray(wp_all[:, 2 * g : 2 * g + 2, :])
        in_maps.append(
            {"xt": xt4, "wqk": wqk, "wv": wv, "wp": wp, "mask": mask_host}
        )
    return in_maps


def kernel(x, W_attn, W_proj):
    from concourse.bass_utils import run_bass_kernel_spmd

    x = np.asarray(x, dtype=np.float32)
    W_attn = np.asarray(W_attn, dtype=np.float32)
    W_proj = np.asarray(W_proj, dtype=np.float32)

    in_maps = _make_in_maps(x, W_attn, W_proj)
    nc = _get_nc()
    res = run_bass_kernel_spmd(nc, in_maps, core_ids=list(range(NCORE)))
    _CACHE["last_result"] = res

    Y = np.empty((B, T, C), dtype=np.float32)
    for b in range(B):
        acc = np.zeros((C, T), dtype=np.float32)
        for g in range(4):
            o = np.asarray(res.results[4 * b + g]["out"], dtype=np.float32)
            acc += o.transpose(0, 2, 1, 3).reshape(C, T)
        Y[b] = acc.T
    return Y


# revision 12
# speedup vs baseline: 1.1982x; 1.0349x over previous
"""Trainium2 8-core Bass kernel for nn_AttentionHPROJ (B=2,T=2048,C=1024,h=16,hd=64).

Sharding: core = 4*b + g owns batch b and heads [4g, 4g+4).
All compute in transposed layout (dout on partitions, tokens moving).

v3: fp16/bf16 matmul dataflow (PSUM accumulation fp32):
  - Q^T/K^T = wqk^T @ x^T in fp16; V natural = x @ Wv^T -> bf16.
  - S^T per head pair packed in one [128,2,512] PSUM tile; h-even on PE rows
    0:64, h-odd on rows 64:128 (concurrent quadrant execution, ~2x).
  - One exp (ACT) per head PAIR over [128,2,width] -> w_pair bf16; causal
    mask multiply on diagonal subtiles (DVE).
  - AV: O^T per head in [65,512] PSUM, ones-column softmax denominator.
  - Normalize: copy denom row to SBUF, reciprocal_approx_fast (the custom op
    NaNs when reading PSUM directly), gpsimd partition_broadcast, DVE mult.
  - c_proj fp16 -> partial Y^T per core, fp16 out; host sums quad + transposes.
  - All proj/c_proj matmul chains run as interleaved PAIRS (alternating PSUM
    banks) so back-to-back matmuls pipeline (~218ns vs ~430ns each).
  - Inputs host-relayouted to SBUF layout: every DMA is [128 part x contiguous
    bytes]; bulk tensors land in 1-2 DMAs; y-out DMAs dispatched from gpsimd.
"""
import sys

sys.path.insert(0, "/opt/trn_rl_repo")

import numpy as np

B, T, C = 2, 2048, 1024
NH, HD = 16, 64
P = 128
NCORE = 8
QC = 512          # q-chunk width
NQC = T // QC     # 4
KSUB = C // P     # 8

_CACHE = {}


def _build():
    import functools
    import concourse.bass as bass
    import concourse.mybir as mybir
    import concourse.tile as tile
    from concourse import bacc

    F32 = mybir.dt.float32
    F32R = mybir.dt.float32r
    F16 = mybir.dt.float16
    BF16 = mybir.dt.bfloat16

    import time as _time
    _t0 = _time.time()
    print("[build] start v3", flush=True)
    nc = bacc.Bacc("TRN2", target_bir_lowering=False, debug=False, num_devices=NCORE)

    xt_ext = nc.dram_tensor("xt", [NQC, P, KSUB, QC], F16, kind="ExternalInput").ap()
    wqk_ext = nc.dram_tensor("wqk", [P, KSUB, 512], F16, kind="ExternalInput").ap()
    wv_ext = nc.dram_tensor("wv", [P, KSUB, 256], F16, kind="ExternalInput").ap()
    wp_ext = nc.dram_tensor("wp", [P, 2, C], F16, kind="ExternalInput").ap()
    mask_ext = nc.dram_tensor("mask", [P, 2, P], BF16, kind="ExternalInput").ap()
    out_ext = nc.dram_tensor("out", [KSUB, NQC, P, QC], F16,
                             kind="ExternalOutput").ap()

    with tile.TileContext(nc) as tc:
        with (
            tc.tile_pool(name="wpool", bufs=1) as wpool,
            tc.tile_pool(name="xpool", bufs=1) as xpool,
            tc.tile_pool(name="qkpool", bufs=1) as qkpool,
            tc.tile_pool(name="vpool", bufs=1) as vpool,
            tc.tile_pool(name="wtile", bufs=4) as wtpool,
            tc.tile_pool(name="opool", bufs=3) as opool,
            tc.tile_pool(name="small", bufs=4) as small,
            tc.tile_pool(name="psS", bufs=2, space="PSUM") as psS,
            tc.tile_pool(name="psO", bufs=2, space="PSUM") as psO,
            tc.tile_pool(name="psY", bufs=2, space="PSUM") as psY,
        ):
            # ---- inputs ----
            wqk_sb = wpool.tile([P, KSUB, 512], F16)
            wv_sb = wpool.tile([P, KSUB, 256], F16)
            xt_t = [xpool.tile([P, KSUB, QC], F16, name=f"xt_{i}", tag=f"xt_{i}")
                    for i in range(NQC)]
            qk_t = [qkpool.tile([P, 4, QC], F16, name=f"qk_{i}", tag=f"qk_{i}")
                    for i in range(NQC)]
            v_t = [vpool.tile([P, 4, 4 * 65], BF16, name=f"v_{i}", tag=f"v_{i}")
                   for i in range(NQC)]
            mask_sb = wpool.tile([P, 2, P], BF16)
            wp_sb = wpool.tile([P, 2, C], F16)

            nc.sync.dma_start(mask_sb[:], mask_ext[:])
            ones_st = wpool.tile([P, 4, 4], BF16)
            nc.vector.memset(ones_st[:], 1.0)
            for i in range(NQC):
                ov = v_t[i].rearrange("p t (h c) -> p t h c", c=65)[:, :, :, 64]
                nc.vector.tensor_copy(out=ov, in_=ones_st[:])
            # early-need inputs on sync, halves for low first-matmul latency
            nc.sync.dma_start(wqk_sb[:, 0:4], wqk_ext[:, 0:4])
            nc.sync.dma_start(xt_t[0][:, 0:4], xt_ext[0, :, 0:4])
            nc.sync.dma_start(wqk_sb[:, 4:8], wqk_ext[:, 4:8])
            nc.sync.dma_start(xt_t[0][:, 4:8], xt_ext[0, :, 4:8])
            nc.scalar.dma_start(wv_sb[:], wv_ext[:])
            nc.scalar.dma_start(wp_sb[:], wp_ext[:])
            nc.gpsimd.dma_start(xt_t[1][:], xt_ext[1])
            nc.scalar.dma_start(xt_t[2][:], xt_ext[2])
            nc.gpsimd.dma_start(xt_t[3][:], xt_ext[3])

            # ---- interleaved-pair projection chains ----
            def qk_pair_step(tc_i, dtA, dtB, st, k):
                if k == 0:
                    st["a"] = psY.tile([P, QC], F32, name=f"pqk{tc_i}{dtA}", tag="Y")
                    st["b"] = psY.tile([P, QC], F32, name=f"pqk{tc_i}{dtB}", tag="Y")
                for ps, dt in ((st["a"], dtA), (st["b"], dtB)):
                    nc.tensor.matmul(
                        ps[:],
                        lhsT=wqk_sb[:, k, dt * P : (dt + 1) * P],
                        rhs=xt_t[tc_i][:, k],
                        start=(k == 0),
                        stop=(k == KSUB - 1),
                    )
                if k == KSUB - 1:
                    nc.vector.tensor_copy(out=qk_t[tc_i][:, dtA, :], in_=st["a"][:])
                    nc.vector.tensor_copy(out=qk_t[tc_i][:, dtB, :], in_=st["b"][:])

            def v_pair_step(tc_i, tlA, tlB, st, k):
                if k == 0:
                    st["a"] = psY.tile([P, 256], F32, name=f"pv{tc_i}{tlA}", tag="Y")
                    st["b"] = psY.tile([P, 256], F32, name=f"pv{tc_i}{tlB}", tag="Y")
                for ps, tl_ in ((st["a"], tlA), (st["b"], tlB)):
                    nc.tensor.matmul(
                        ps[:],
                        lhsT=xt_t[tc_i][:, k, tl_ * P : (tl_ + 1) * P],
                        rhs=wv_sb[:, k, :],
                        start=(k == 0),
                        stop=(k == KSUB - 1),
                    )
                if k == KSUB - 1:
                    for ps, tl_ in ((st["a"], tlA), (st["b"], tlB)):
                        vdst = v_t[tc_i].rearrange(
                            "p t (h c) -> p t h c", c=65)[:, tl_, :, 0:64]
                        nc.vector.tensor_copy(
                            out=vdst, in_=ps.rearrange("p (h d) -> p h d", d=64)
                        )

            def pair_unit(fn, *args):
                st = {}
                return [functools.partial(fn, *args, st, k) for k in range(KSUB)]

            def cproj_steps(qc, o_t, dtA, dtB, use_act=False):
                st = {}

                def step(pr):
                    if pr == 0:
                        st["a"] = psY.tile([P, QC], F32, name=f"py{qc}{dtA}", tag="Y")
                        st["b"] = psY.tile([P, QC], F32, name=f"py{qc}{dtB}", tag="Y")
                    for ps, dt in ((st["a"], dtA), (st["b"], dtB)):
                        nc.tensor.matmul(
                            ps[:],
                            lhsT=wp_sb[:, pr, dt * P : (dt + 1) * P],
                            rhs=o_t[:, pr, :],
                            start=(pr == 0),
                            stop=(pr == 1),
                        )
                    if pr == 1:
                        for i, (ps, dt) in enumerate(((st["a"], dtA), (st["b"], dtB))):
                            y_t = small.tile([P, QC], F16, name="y_t", tag="YS")
                            if use_act and i == 0:
                                nc.scalar.copy(out=y_t[:], in_=ps[:])
                            else:
                                nc.vector.tensor_copy(out=y_t[:], in_=ps[:])
                            nc.gpsimd.dma_start(out_ext[dt, qc], y_t[:])

                return [functools.partial(step, pr) for pr in range(2)]

            def phase1_steps(tc_i):
                return (pair_unit(qk_pair_step, tc_i, 0, 2)
                        + pair_unit(qk_pair_step, tc_i, 1, 3)
                        + pair_unit(v_pair_step, tc_i, 0, 1)
                        + pair_unit(v_pair_step, tc_i, 2, 3))

            # ---- attention ----
            def attn_pair(qc, hp, consume):  # consume() called once per slot
                h0, h1 = 2 * hp, 2 * hp + 1
                qpl, kpl = hp, 2 + hp
                nkt = 4 * qc + 4
                po0 = psO.tile([65, QC], F32, name=f"po_{qc}_{h0}", tag="O")
                po1 = psO.tile([65, QC], F32, name=f"po_{qc}_{h1}", tag="O")
                pending = None

                def emit_av(w_pair, kb, kl, csl, kt):
                    nc.tensor.matmul(
                        po0[:, csl],
                        lhsT=v_t[kb][:, kl, 65 * h0 : 65 * h0 + 65],
                        rhs=w_pair[:, 0, csl],
                        start=(kt == 0),
                        stop=(kt == nkt - 1),
                    )
                    nc.tensor.matmul(
                        po1[:, csl],
                        lhsT=v_t[kb][:, kl, 65 * h1 : 65 * h1 + 65],
                        rhs=w_pair[:, 1, csl],
                        start=(kt == 0),
                        stop=(kt == nkt - 1),
                    )

                for kt in range(nkt):
                    j = kt - 4 * qc
                    c0 = max(0, j) * P
                    csl = slice(c0, QC)
                    kb, kl = kt // 4, kt % 4
                    ps_pair = psS.tile([P, 2, QC], F32,
                                       name=f"ps_s_{qc}_{hp}_{kt}", tag="S")
                    nc.tensor.matmul(
                        ps_pair[:, 0, csl],
                        lhsT=qk_t[kb][0:64, kpl, kl * P : (kl + 1) * P],
                        rhs=qk_t[qc][0:64, qpl, c0:QC],
                        start=True, stop=True,
                    )
                    nc.tensor.matmul(
                        ps_pair[:, 1, csl],
                        lhsT=qk_t[kb][64:128, kpl, kl * P : (kl + 1) * P],
                        rhs=qk_t[qc][64:128, qpl, c0:QC],
                        start=True, stop=True,
                    )
                    w_pair = wtpool.tile([P, 2, QC], BF16, name="w_pair", tag="W")
                    nc.scalar.activation(
                        w_pair[:, :, csl], ps_pair[:, :, csl],
                        mybir.ActivationFunctionType.Exp,
                    )
                    if j >= 0:
                        nc.vector.tensor_tensor(
                            out=w_pair[:, :, c0 : c0 + P],
                            in0=w_pair[:, :, c0 : c0 + P],
                            in1=mask_sb[:],
                            op=mybir.AluOpType.mult,
                        )
                    if pending is not None:
                        consume()
                        emit_av(*pending)
                    pending = (w_pair, kb, kl, csl, kt)
                emit_av(*pending)
                return po0, po1

            def finish_pair(o_t, hp, po0, po1):
                dens, recs, rbcs = [], [], []
                for po in (po0, po1):
                    den = small.tile([1, QC], F32, name="den", tag="DN")
                    nc.vector.tensor_copy(out=den[:], in_=po[64:65, :])
                    dens.append(den)
                for den in dens:
                    rec = small.tile([1, QC], F32, name="rec", tag="R")
                    nc.vector.reciprocal_approx_fast(out=rec[:], in_=den[:])
                    recs.append(rec)
                for rec in recs:
                    rbc = small.tile([64, QC], F32, name="rbc", tag="RB")
                    nc.gpsimd.partition_broadcast(rbc[:], rec[:])
                    rbcs.append(rbc)
                for po, pb, rbc in ((po0, 0, rbcs[0]), (po1, 64, rbcs[1])):
                    nc.vector.tensor_tensor(
                        out=o_t[pb : pb + 64, hp, :],
                        in0=po[0:64, :],
                        in1=rbc[:],
                        op=mybir.AluOpType.mult,
                    )

            def attn_block(qc, s_hp0, s_rest):
                """s_hp0 drains inside the hp=0 pair (pre-AV slots); s_rest
                spreads across BOTH pairs and drains at block end."""
                o_t = opool.tile([P, 2, QC], F16, name="o_t", tag="OT")
                nkt = 4 * qc + 4
                nslots = 2 * (nkt - 1)
                rest = list(s_rest)
                rdone = [0]
                slot = [0]

                def consume_rest():
                    slot[0] += 1
                    want = (slot[0] * len(rest)) // max(1, nslots)
                    while rdone[0] < min(want, len(rest)):
                        rest[rdone[0]]()
                        rdone[0] += 1

                for hp in range(2):
                    if hp == 0 and s_hp0:
                        h0stream = list(s_hp0)
                        h0done = [0]

                        def consume(kt=None, h0stream=h0stream, h0done=h0done):
                            slot[0] += 1
                            want = (slot[0] * len(h0stream)) // (nkt - 1)
                            while h0done[0] < min(want, len(h0stream)):
                                h0stream[h0done[0]]()
                                h0done[0] += 1
                    else:
                        consume = consume_rest
                    po0, po1 = attn_pair(qc, hp, consume)
                    if hp == 0 and s_hp0:
                        for f in h0stream[h0done[0]:]:
                            f()
                        slot[0] = nkt - 1
                    finish_pair(o_t, hp, po0, po1)
                for f in rest[rdone[0]:]:
                    f()
                return o_t

            # ---- schedule ----
            # prologue: Q/K pair0 of chunk 0 only; V chunk 0 + Q/K pair1
            # stream into block 0's hp0 pre-AV slots
            for f in pair_unit(qk_pair_step, 0, 0, 2):
                f()
            o_prev = None
            o_pp = None
            for qc in range(NQC):
                if qc == 0:
                    s_hp0 = (pair_unit(v_pair_step, 0, 0, 1)
                             + pair_unit(v_pair_step, 0, 2, 3)
                             + pair_unit(qk_pair_step, 0, 1, 3))
                    s_rest = phase1_steps(1)
                else:
                    s_hp0 = []
                    s_rest = []
                    if qc >= 2 and o_pp is not None:
                        for dtA in (0, 2, 4, 6):
                            s_rest += cproj_steps(qc - 2, o_pp, dtA, dtA + 1)
                    if qc == 3 and o_prev is not None:
                        for dtA in (0, 2, 4, 6):
                            s_rest += cproj_steps(qc - 1, o_prev, dtA, dtA + 1)
                    if qc + 1 < NQC:
                        s_rest += phase1_steps(qc + 1)
                o_pp = o_prev
                o_prev = attn_block(qc, s_hp0, s_rest)
            for dtA in (0, 2, 4, 6):
                for f in cproj_steps(NQC - 1, o_prev, dtA, dtA + 1, use_act=True):
                    f()

    print(f"[build] traced+scheduled {_time.time()-_t0:.1f}s", flush=True)
    nc.compile()
    print(f"[build] compiled {_time.time()-_t0:.1f}s", flush=True)
    return nc


def _get_nc():
    if "nc" not in _CACHE:
        _CACHE["nc"] = _build()
    return _CACHE["nc"]


def _make_in_maps(x, W_attn, W_proj):
    import ml_dtypes
    Wp = W_proj.reshape(NH, C, HD)  # [head, dout, d]
    A = Wp.reshape(8, 2, C, HD)     # [pair, hl, dout, d]
    wp_all = np.ascontiguousarray(
        A.transpose(1, 3, 0, 2).reshape(P, 8, C)
    ).astype(np.float16)
    m = np.triu(np.ones([P, P], dtype=np.float32))
    mask_host = np.ascontiguousarray(
        np.broadcast_to(m[:, None, :], (P, 2, P))
    ).astype(ml_dtypes.bfloat16)

    def part_major(a):  # [KSUB*P, M] -> [P, KSUB, M]
        return np.ascontiguousarray(
            a.reshape(KSUB, P, a.shape[1]).transpose(1, 0, 2)
        )

    in_maps = []
    for core in range(NCORE):
        b, g = core // 4, core % 4
        xt = x[b].T.astype(np.float16)  # [C, T]
        # [NQC, P, KSUB, QC]
        xt4 = np.ascontiguousarray(
            xt.reshape(KSUB, P, NQC, QC).transpose(2, 1, 0, 3)
        )
        Wq = W_attn[256 * g : 256 * (g + 1)]
        Wk = W_attn[C + 256 * g : C + 256 * (g + 1)]
        Wv = W_attn[2 * C + 256 * g : 2 * C + 256 * (g + 1)]
        wqk = part_major(
            np.concatenate([Wq, Wk], 0).T.astype(np.float16))  # [P,KSUB,512]
        wv = part_major(Wv.T.astype(np.float16))               # [P,KSUB,256]
        wp = np.ascontiguousarray(wp_all[:, 2 * g : 2 * g + 2, :])
        in_maps.append(
            {"xt": xt4, "wqk": wqk, "wv": wv, "wp": wp, "mask": mask_host}
        )
    return in_maps


def kernel(x, W_attn, W_proj):
    from concourse.bass_utils import run_bass_kernel_spmd

    x = np.asarray(x, dtype=np.float32)
    W_attn = np.asarray(W_attn, dtype=np.float32)
    W_proj = np.asarray(W_proj, dtype=np.float32)

    in_maps = _make_in_maps(x, W_attn, W_proj)
    nc = _get_nc()
    res = run_bass_kernel_spmd(nc, in_maps, core_ids=list(range(NCORE)))
    _CACHE["last_result"] = res

    Y = np.empty((B, T, C), dtype=np.float32)
    for b in range(B):
        acc = np.zeros((C, T), dtype=np.float32)
        for g in range(4):
            o = np.asarray(res.results[4 * b + g]["out"], dtype=np.float32)
            acc += o.transpose(0, 2, 1, 3).reshape(C, T)
        Y[b] = acc.T
    return Y
